# revision 30
# baseline (speedup 1.0000x reference)
"""2-layer GCN encoder (PyG GCNConv semantics) on 8 Trainium2 NeuronCores.

Distribution (per sharding_hint): nodes are sharded across the 8 cores
(12544 destination rows each); W1/W2 replicated; edges routed by
destination core. Per layer the aggregation A_hat @ H runs as:
  - 128-wide bf16 feature table (q~ = dis*(x@W1), or h~ = dis*h) in each
    core's HBM, built locally (layer 1) or via AllGather of the per-core
    shards (layer 2 input),
  - GPSIMD dma_gather of the per-edge source rows (256B descriptors,
    int16 indices against 4 table banks of 25088 rows),
  - segment-sum by destination as one-hot S-matrix matmuls on the
    TensorEngine accumulating in PSUM (S built on the VectorEngine from
    iota+is_equal against host-prepared dest-local ids),
  - epilogue on Vector/Scalar engines (self-loop add, deg^-1/2 scaling,
    bias, relu), final @W2 via PE transpose + matmul.

The host side only prepares index/layout data: deg^-1/2, a node
permutation that balances (tile, src-bank) gather-budget groups, int16
gather indices, dest-local ids, and transposed/bf16-cast inputs.

The edge factorization used on device: with h~ = dis ⊙ h,
  (A_hat @ h)[d] = dis[d] * ( sum_{e: dst=d} h~[src_e] + h~[d] ).
"""
import os
import sys
import time

_STAGE = int(os.environ.get("GCN_STAGE", "3"))  # 1=Q+AG1, 2=+L1+AG2, 3=full

for _p in ("/opt/trn_rl_repo/concourse", "/opt/trn_rl_repo"):
    if _p not in sys.path:
        sys.path.insert(0, _p)

import numpy as np
import ml_dtypes

N = 100000          # real nodes
E = 640000          # edges
IN = 16
F = 128             # hidden/out feature dim
NCORES = 8
SHARD = 12544       # nodes per core (98 tiles of 128)
NP = NCORES * SHARD  # 100352 padded nodes
T = 98              # dest tiles per core
NBANKS = 4
BANK = NP // NBANKS  # 25088 table rows per gather bank
LTB = 256           # slot budget per (tile, src-bank) group
G = 4               # tiles per gather group (4*LTB = 1024 = max idx/call)
GROUPS = [(g * G, min(G, T - g * G)) for g in range((T + G - 1) // G)]
GBASE = [0]
for _, _nt in GROUPS:
    GBASE.append(GBASE[-1] + NBANKS * _nt * LTB)
SLOTS = GBASE[-1]             # 100352 slots per core per layer
BF = ml_dtypes.bfloat16

_CACHE = {}


def _build_device():
    from concourse import bacc, tile, mybir

    BF16 = mybir.dt.bfloat16
    F32 = mybir.dt.float32
    I16 = mybir.dt.int16

    nc = bacc.Bacc(None, target_bir_lowering=False, num_devices=NCORES,
                   name="gcn8", num_swdge_queues=2)

    xT = nc.declare_dram_parameter("xT", [IN, SHARD], BF16, isOutput=False)
    xTf = nc.declare_dram_parameter("xTf", [IN, NP], BF16, isOutput=False)
    disF = nc.declare_dram_parameter("disF", [128, NP // 128], F32,
                                     isOutput=False)
    w1 = nc.declare_dram_parameter("W1", [IN, F], BF16, isOutput=False)
    w2 = nc.declare_dram_parameter("W2", [F, F], BF16, isOutput=False)
    b1b = nc.declare_dram_parameter("b1b", [128, F], BF16, isOutput=False)
    b2c = nc.declare_dram_parameter("b2c", [F, 1], F32, isOutput=False)
    disc = nc.declare_dram_parameter("disc", [128, T], F32, isOutput=False)
    idx = nc.declare_dram_parameter("idx", [16, SLOTS // 16], I16, isOutput=False)
    dloc = nc.declare_dram_parameter("dloc", [128, SLOTS // 128], BF16,
                                     isOutput=False)
    outT = nc.declare_dram_parameter("outT", [F, SHARD], F32, isOutput=True)

    q_bank = [nc.dram_tensor(f"q_bank{b}", [BANK, F], BF16)
              for b in range(NBANKS)]
    h_shard = nc.dram_tensor("h_shard", [SHARD, F], BF16)
    h_full = nc.dram_tensor("h_full", [NP, F], BF16)

    groups = [list(range(NCORES))]

    with tile.TileContext(nc) as tc:
        with (
            tc.tile_pool(name="sb", bufs=1) as sb,
            tc.tile_pool(name="msg", bufs=2) as msgp,
            tc.tile_pool(name="spool", bufs=2) as spool,
            tc.tile_pool(name="ep", bufs=3) as ep,
            tc.tile_pool(name="ppq", bufs=2, space="PSUM") as ppq,
            tc.tile_pool(name="ppa", bufs=2, space="PSUM") as ppa,
            tc.tile_pool(name="ppt", bufs=2, space="PSUM") as ppt,
        ):
            # ---------- load constants / metadata ----------
            xT_t = sb.tile([IN, SHARD], BF16)
            disF_t = sb.tile([128, NP // 128], F32)
            nc.sync.dma_start(disF_t[:], disF[:])
            w1_t = sb.tile([IN, F], BF16)
            w2_t = sb.tile([F, F], BF16)
            b1_t = sb.tile([128, F], BF16)
            b2_t = sb.tile([F, 1], F32)
            disf_t = sb.tile([128, T], F32)
            dis_t = sb.tile([128, T], BF16)
            dloc_t = sb.tile([128, SLOTS // 128], BF16)
            idx_t = sb.tile([128, SLOTS // 16], I16)
            nc.sync.dma_start(xT_t[:], xT[:])
            nc.sync.dma_start(w1_t[:], w1[:])
            nc.sync.dma_start(w2_t[:], w2[:])
            nc.sync.dma_start(b1_t[:], b1b[:])
            nc.sync.dma_start(b2_t[:], b2c[:])
            nc.sync.dma_start(disf_t[:], disc[:])
            nc.vector.tensor_copy(dis_t[:], disf_t[:])
            nc.sync.dma_start(dloc_t[:], dloc[:])
            for g8 in range(8):
                nc.sync.dma_start(idx_t[g8 * 16:(g8 + 1) * 16, :], idx[:])

            iota_t = sb.tile([128, 128], BF16)
            nc.gpsimd.iota(iota_t[:], pattern=[[1, 128]], base=0,
                           channel_multiplier=0,
                           allow_small_or_imprecise_dtypes=True)
            one_t = sb.tile([128, 128], BF16)
            nc.gpsimd.memset(one_t[:], 1.0)
            ident = sb.tile([128, 128], BF16)
            nc.gpsimd.affine_select(
                ident[:], one_t[:], pattern=[[-1, 128]],
                compare_op=mybir.AluOpType.is_equal, fill=0.0,
                base=0, channel_multiplier=1)
            zb = sb.tile([128, 1], F32)
            nc.gpsimd.memset(zb[:], 0.0)

            qself = sb.tile([128, T, F], BF16)
            hself = sb.tile([128, T, F], BF16)

            # ---------- stage Q: every core computes the FULL q~ table
            # q~ = dis * (x @ W1), 784 chunk matmuls -> per-bank HBM tables.
            # No collective: replicated compute is cheaper than AllGather.
            TPB = (NP // 128) // NBANKS  # 196 tiles per bank
            for seg in range(NCORES):
                xs = msgp.tile([IN, SHARD], BF16, tag="xs")
                nc.sync.dma_start(xs[:], xTf[:, seg * SHARD:(seg + 1) * SHARD])
                for t in range(T):
                    cch = seg * T + t
                    ps = ppq.tile([128, F], F32, tag="qps")
                    nc.tensor.matmul(ps[:], xs[:, t * 128:(t + 1) * 128],
                                     w1_t[:], start=True, stop=True)
                    qt = ep.tile([128, F], BF16, tag="qt")
                    nc.vector.tensor_tensor(
                        qt[:], ps[:],
                        disF_t[:, cch:cch + 1].broadcast_to([128, F]),
                        mybir.AluOpType.mult)
                    b = cch // TPB
                    lt = cch % TPB
                    nc.sync.dma_start(q_bank[b][lt * 128:(lt + 1) * 128, :],
                                      qt[:])

            # own-shard q~ rows kept in SBUF for the layer-1 self-loop term
            for t in range(T):
                ps = ppq.tile([128, F], F32, tag="qps")
                nc.tensor.matmul(ps[:], xT_t[:, t * 128:(t + 1) * 128], w1_t[:],
                                 start=True, stop=True)
                nc.vector.tensor_tensor(
                    qself[:, t, :], ps[:],
                    disf_t[:, t:t + 1].broadcast_to([128, F]),
                    mybir.AluOpType.mult)

            # ---------- aggregation layers ----------
            qcnt = [0]

            def agg_layer(bank_ap, self_t, out_cb):
                """out_cb(t, g2_tile_bf16) consumes dis*(psum+self) per tile."""
                for g, (t0, nt) in enumerate(GROUPS):
                    nch = NBANKS * nt * (LTB // 128)
                    mb = msgp.tile([128, nch, F], BF16, tag="mb")
                    s0 = GBASE[g]
                    nidx = nt * LTB
                    for b in range(NBANKS):
                        c0 = (s0 + b * nidx) // 16
                        nc.gpsimd.dma_gather(
                            mb[:, b * (nidx // 128):(b + 1) * (nidx // 128), :],
                            bank_ap(b),
                            idx_t[:, c0:c0 + nidx // 16],
                            nidx, nidx, F, queue_num=qcnt[0] % 2)
                        qcnt[0] += 1
                    S_t = spool.tile([128, nch, 128], BF16, tag="S")
                    nc.vector.tensor_tensor(
                        S_t[:],
                        dloc_t[:, s0 // 128:s0 // 128 + nch]
                            .unsqueeze(-1).broadcast_to([128, nch, 128]),
                        iota_t[:].unsqueeze(1).broadcast_to([128, nch, 128]),
                        mybir.AluOpType.is_equal)
                    for ti in range(nt):
                        t = t0 + ti
                        gps = ppa.tile([128, F], F32, tag="gps")
                        for b in range(NBANKS):
                            for k in range(LTB // 128):
                                ch = b * (nt * LTB // 128) + ti * (LTB // 128) + k
                                nc.tensor.matmul(
                                    gps[:], S_t[:, ch, :], mb[:, ch, :],
                                    start=(b == 0 and k == 0),
                                    stop=(b == NBANKS - 1 and k == LTB // 128 - 1))
                        t1 = ep.tile([128, F], BF16, tag="t1")
                        nc.vector.tensor_copy(t1[:], gps[:])
                        nc.vector.tensor_tensor(t1[:], t1[:], self_t[:, t, :],
                                                mybir.AluOpType.add)
                        g2 = ep.tile([128, F], BF16, tag="g2")
                        nc.vector.tensor_tensor(
                            g2[:], t1[:],
                            dis_t[:, t:t + 1].broadcast_to([128, F]),
                            mybir.AluOpType.mult)
                        out_cb(t, g2)

            # ----- layer 1: table q~, epilogue -> h~ shard + allgather
            def l1_out(t, g2):
                t3 = ep.tile([128, F], BF16, tag="t3")
                nc.vector.tensor_tensor(t3[:], g2[:], b1_t[:],
                                        mybir.AluOpType.add)
                hr = ep.tile([128, F], BF16, tag="hr")
                nc.scalar.activation(hr[:], t3[:],
                                     mybir.ActivationFunctionType.Relu,
                                     bias=zb[:])
                nc.vector.tensor_tensor(
                    hself[:, t, :], hr[:],
                    dis_t[:, t:t + 1].broadcast_to([128, F]),
                    mybir.AluOpType.mult)
                nc.sync.dma_start(h_shard[t * 128:(t + 1) * 128, :],
                                  hself[:, t, :])

            agg_layer(lambda b: q_bank[b][:], qself, l1_out)

            nc.gpsimd.collective_compute(
                "AllGather", mybir.AluOpType.bypass, replica_groups=groups,
                ins=[h_shard[:].opt()], outs=[h_full[:].opt()])

            # ----- layer 2: table h~, epilogue -> transpose, @W2, +b2
            def l2_out(t, g2):
                tps = ppt.tile([128, 128], BF16, tag="tps")
                nc.tensor.transpose(tps[:], g2[:], ident[:])
                g2T = ep.tile([128, 128], BF16, tag="g2T")
                nc.vector.tensor_copy(g2T[:], tps[:])
                ops = ppt.tile([128, 128], F32, tag="ops")
                nc.tensor.matmul(ops[:], w2_t[:], g2T[:], start=True, stop=True)
                ot = ep.tile([128, 128], F32, tag="ot")
                nc.vector.tensor_tensor(ot[:], ops[:],
                                        b2_t[:].broadcast_to([128, 128]),
                                        mybir.AluOpType.add)
                nc.sync.dma_start(outT[:, t * 128:(t + 1) * 128], ot[:])

            agg_layer(lambda b: h_full[b * BANK:(b + 1) * BANK, :],
                      hself, l2_out)

    nc.compile()
    return nc


def _get_runner():
    if "runner" not in _CACHE:
        from runner_embed import SpmdRunner
        nc = _build_device()
        _CACHE["nc"] = nc
        _CACHE["runner"] = SpmdRunner(nc, NCORES)
    return _CACHE["runner"]


# ---------------------------------------------------------------------------
# host-side index preparation
# ---------------------------------------------------------------------------

def _prep(edge_index):
    """Returns (perm, pos, dis, per-core metadata arrays).

    perm[i] = original node id at permuted position i; pos = inverse.
    The permutation keeps every node inside its original quarter
    (= gather bank), so bank assignments are permutation-independent.
    Within each quarter, nodes are dealt snake-wise by in-degree across
    the quarter's 196 tiles to balance (tile, src-bank) group loads.
    """
    row = np.asarray(edge_index[0], dtype=np.int64)
    col = np.asarray(edge_index[1], dtype=np.int64)

    deg = np.bincount(col, minlength=NP).astype(np.float32) + 1.0
    dis = 1.0 / np.sqrt(deg)

    src_bank = col // BANK  # fixed: sources keep their quarter

    # in-degree per node (for balancing)
    indeg = np.bincount(row, minlength=NP)

    perm = np.empty(NP, dtype=np.int64)
    TPQ = 2 * T  # tiles per quarter (2 cores per bank-quarter)
    for q in range(NBANKS):
        nodes = np.arange(q * BANK, (q + 1) * BANK)
        order = np.argsort(-indeg[nodes], kind="stable")
        nodes = nodes[order]
        # snake deal across TPQ tiles
        k = np.arange(BANK)
        rnd, pos_in_rnd = k // TPQ, k % TPQ
        tile_of = np.where(rnd % 2 == 0, pos_in_rnd, TPQ - 1 - pos_in_rnd)
        slot_of = rnd
        dest_pos = q * BANK + tile_of * 128 + slot_of
        perm[dest_pos] = nodes
    pos = np.empty(NP, dtype=np.int64)
    pos[perm] = np.arange(NP)

    prow = pos[row]           # permuted dest positions
    pcol = pos[col]           # permuted src positions
    core = prow // SHARD
    tilec = (prow % SHARD) // 128

    # group key (core, tile, bank) and slot packing
    gkey = (core * T + tilec) * NBANKS + src_bank
    loads = np.bincount(gkey, minlength=NCORES * T * NBANKS)
    mx = loads.max()
    if mx > LTB:
        raise RuntimeError(f"gather budget overflow: max group load {mx} > {LTB}")

    order = np.argsort(gkey, kind="stable")
    e_sorted = order
    gk_sorted = gkey[order]
    # rank within group
    starts = np.zeros(NCORES * T * NBANKS + 1, dtype=np.int64)
    np.cumsum(loads, out=starts[1:])
    rank = np.arange(E) - starts[gk_sorted]

    # slot id within the core: layout [group g][bank b][tile-in-group][LTB]
    tile_of = gk_sorted // NBANKS % T
    t_in_g = tile_of % G
    g_of = tile_of // G
    b_of = gk_sorted % NBANKS
    nt_of = np.minimum(G, T - g_of * G)
    gbase = np.asarray(GBASE[:-1], dtype=np.int64)
    slot = gbase[g_of] + b_of * (nt_of * LTB) + t_in_g * LTB + rank
    core_of = gk_sorted // (T * NBANKS)

    idx_arr = np.zeros((NCORES, 16, SLOTS // 16), dtype=np.int16)
    dl_arr = np.full((NCORES, 128, SLOTS // 128), 255.0, dtype=BF)

    idx_val = (pcol[e_sorted] - src_bank[e_sorted] * BANK).astype(np.int16)
    dl_val = (prow[e_sorted] % 128).astype(np.float32)
    idx_arr[core_of, slot % 16, slot // 16] = idx_val
    dl_arr[core_of, slot % 128, slot // 128] = dl_val.astype(BF)

    dis_perm = dis[perm]      # dis by permuted position
    return perm, pos, dis, dis_perm, idx_arr, dl_arr


def kernel(x, W1, b1, W2, b2, edge_index):
    x = np.asarray(x, dtype=np.float32)
    W1 = np.asarray(W1, dtype=np.float32)
    b1 = np.asarray(b1, dtype=np.float32)
    W2 = np.asarray(W2, dtype=np.float32)
    b2 = np.asarray(b2, dtype=np.float32)
    edge_index = np.asarray(edge_index)

    perm, pos, dis, dis_perm, idx_arr, dl_arr = _prep(edge_index)

    xp = np.zeros((NP, IN), dtype=np.float32)
    xp[:N] = x
    xp = xp[perm]             # permuted node order

    xTf_full = np.ascontiguousarray(xp.T).astype(BF)
    disF_full = np.ascontiguousarray(
        dis_perm.reshape(NP // 128, 128).T).astype(np.float32)

    in_maps = []
    for c in range(NCORES):
        sl = slice(c * SHARD, (c + 1) * SHARD)
        in_maps.append({
            "xT": np.ascontiguousarray(xp[sl].T).astype(BF),
            "xTf": xTf_full,
            "disF": disF_full,
            "W1": W1.astype(BF),
            "W2": W2.astype(BF),
            "b1b": np.tile(b1[None, :], (128, 1)).astype(BF),
            "b2c": b2[:, None].astype(np.float32),
            "disc": np.ascontiguousarray(
                dis_perm[sl].reshape(T, 128).T).astype(np.float32),
            "idx": idx_arr[c],
            "dloc": dl_arr[c],
        })

    r = _get_runner()
    ci = r.prep_inputs(in_maps)
    out = r.run(ci)
    res = r.results(out)

    full = np.concatenate([res[c]["outT"] for c in range(NCORES)], axis=1)
    return np.ascontiguousarray(full.T[pos[:N]]).astype(np.float32)


# revision 38
# speedup vs baseline: 1.3963x; 1.3963x over previous
"""2-layer GCN encoder (PyG GCNConv semantics) on 8 Trainium2 NeuronCores.

Distribution (per sharding_hint): nodes are sharded across the 8 cores
(12544 destination rows each); W1/W2 replicated; edges routed by
destination core. Per layer the aggregation A_hat @ H runs as:
  - 128-wide bf16 feature table (q~ = dis*(x@W1), or h~ = dis*h) in each
    core's HBM, built locally (layer 1) or via AllGather of the per-core
    shards (layer 2 input),
  - GPSIMD dma_gather of the per-edge source rows (256B descriptors,
    int16 indices against 4 table banks of 25088 rows),
  - segment-sum by destination as one-hot S-matrix matmuls on the
    TensorEngine accumulating in PSUM (S built on the VectorEngine from
    iota+is_equal against host-prepared dest-local ids),
  - epilogue on Vector/Scalar engines (self-loop add, deg^-1/2 scaling,
    bias, relu), final @W2 via PE transpose + matmul.

The host side only prepares index/layout data: deg^-1/2, a node
permutation that balances (tile, src-bank) gather-budget groups, int16
gather indices, dest-local ids, and transposed/bf16-cast inputs.

The edge factorization used on device: with h~ = dis ⊙ h,
  (A_hat @ h)[d] = dis[d] * ( sum_{e: dst=d} h~[src_e] + h~[d] ).
"""
import os
import sys
import time

_STAGE = int(os.environ.get("GCN_STAGE", "3"))  # 1=Q+AG1, 2=+L1+AG2, 3=full

for _p in ("/opt/trn_rl_repo/concourse", "/opt/trn_rl_repo"):
    if _p not in sys.path:
        sys.path.insert(0, _p)

import numpy as np
import ml_dtypes

N = 100000          # real nodes
E = 640000          # edges
IN = 16
F = 128             # hidden/out feature dim
NCORES = 8
SHARD = 12544       # nodes per core (98 tiles of 128)
NP = NCORES * SHARD  # 100352 padded nodes
T = 98              # dest tiles per core
NBANKS = 4
BANK = NP // NBANKS  # 25088 table rows per gather bank
LTB = 256           # slot budget per (tile, src-bank) group
G = 4               # tiles per gather group (4*LTB = 1024 = max idx/call)
GROUPS = [(g * G, min(G, T - g * G)) for g in range((T + G - 1) // G)]
GBASE = [0]
for _, _nt in GROUPS:
    GBASE.append(GBASE[-1] + NBANKS * _nt * LTB)
SLOTS = GBASE[-1]             # 100352 slots per core per layer
BF = ml_dtypes.bfloat16

_CACHE = {}


def _build_device():
    from concourse import bacc, tile, mybir

    BF16 = mybir.dt.bfloat16
    F32 = mybir.dt.float32
    I16 = mybir.dt.int16

    nc = bacc.Bacc(None, target_bir_lowering=False, num_devices=NCORES,
                   name="gcn8", num_swdge_queues=2)

    xT = nc.declare_dram_parameter("xT", [IN, SHARD], BF16, isOutput=False)
    w1 = nc.declare_dram_parameter("W1", [IN, F], BF16, isOutput=False)
    w2 = nc.declare_dram_parameter("W2", [F, F], BF16, isOutput=False)
    b1b = nc.declare_dram_parameter("b1b", [128, F], BF16, isOutput=False)
    b2c = nc.declare_dram_parameter("b2c", [F, 1], F32, isOutput=False)
    disc = nc.declare_dram_parameter("disc", [128, T], F32, isOutput=False)
    idx = nc.declare_dram_parameter("idx", [16, SLOTS // 16], I16, isOutput=False)
    dloc = nc.declare_dram_parameter("dloc", [128, SLOTS // 128], BF16,
                                     isOutput=False)
    outT = nc.declare_dram_parameter("outT", [F, SHARD], F32, isOutput=True)

    q_shard = nc.dram_tensor("q_shard", [SHARD, F], BF16)
    q_full = nc.dram_tensor("q_full", [NP, F], BF16)
    h_shard = nc.dram_tensor("h_shard", [SHARD, F], BF16)
    h_full = nc.dram_tensor("h_full", [NP, F], BF16)

    groups = [list(range(NCORES))]

    with tile.TileContext(nc) as tc:
        with (
            tc.tile_pool(name="sb", bufs=1) as sb,
            tc.tile_pool(name="msg", bufs=2) as msgp,
            tc.tile_pool(name="spool", bufs=2) as spool,
            tc.tile_pool(name="ep", bufs=3) as ep,
            tc.tile_pool(name="ppq", bufs=2, space="PSUM") as ppq,
            tc.tile_pool(name="ppa", bufs=2, space="PSUM") as ppa,
            tc.tile_pool(name="ppt", bufs=2, space="PSUM") as ppt,
        ):
            # ---------- load constants / metadata ----------
            xT_t = sb.tile([IN, SHARD], BF16)
            w1_t = sb.tile([IN, F], BF16)
            w2_t = sb.tile([F, F], BF16)
            b1_t = sb.tile([128, F], BF16)
            b2_t = sb.tile([F, 1], F32)
            disf_t = sb.tile([128, T], F32)
            dis_t = sb.tile([128, T], BF16)
            dloc_t = sb.tile([128, SLOTS // 128], BF16)
            idx_t = sb.tile([128, SLOTS // 16], I16)
            nc.sync.dma_start(xT_t[:], xT[:])
            nc.sync.dma_start(w1_t[:], w1[:])
            nc.sync.dma_start(w2_t[:], w2[:])
            nc.sync.dma_start(b1_t[:], b1b[:])
            nc.sync.dma_start(b2_t[:], b2c[:])
            nc.sync.dma_start(disf_t[:], disc[:])
            nc.vector.tensor_copy(dis_t[:], disf_t[:])
            nc.sync.dma_start(dloc_t[:], dloc[:])
            for g8 in range(8):
                nc.sync.dma_start(idx_t[g8 * 16:(g8 + 1) * 16, :], idx[:])

            iota_t = sb.tile([128, 128], BF16)
            nc.gpsimd.iota(iota_t[:], pattern=[[1, 128]], base=0,
                           channel_multiplier=0,
                           allow_small_or_imprecise_dtypes=True)
            one_t = sb.tile([128, 128], BF16)
            nc.gpsimd.memset(one_t[:], 1.0)
            ident = sb.tile([128, 128], BF16)
            nc.gpsimd.affine_select(
                ident[:], one_t[:], pattern=[[-1, 128]],
                compare_op=mybir.AluOpType.is_equal, fill=0.0,
                base=0, channel_multiplier=1)
            zb = sb.tile([128, 1], F32)
            nc.gpsimd.memset(zb[:], 0.0)

            qself = sb.tile([128, T, F], BF16)
            hself = sb.tile([128, T, F], BF16)

            # ---------- stage Q: q~ shard = dis * (x @ W1), then AllGather
            for t in range(T):
                ps = ppq.tile([128, F], F32, tag="qps")
                nc.tensor.matmul(ps[:], xT_t[:, t * 128:(t + 1) * 128], w1_t[:],
                                 start=True, stop=True)
                nc.vector.tensor_tensor(
                    qself[:, t, :], ps[:],
                    disf_t[:, t:t + 1].broadcast_to([128, F]),
                    mybir.AluOpType.mult)
                nc.sync.dma_start(q_shard[t * 128:(t + 1) * 128, :],
                                  qself[:, t, :])

            nc.gpsimd.collective_compute(
                "AllGather", mybir.AluOpType.bypass, replica_groups=groups,
                ins=[q_shard[:].opt()], outs=[q_full[:].opt()])

            # ---------- aggregation layers ----------
            qcnt = [0]

            def agg_layer(bank_ap, self_t, out_cb):
                """out_cb(t, g2_tile_bf16) consumes dis*(psum+self) per tile."""
                for g, (t0, nt) in enumerate(GROUPS):
                    nch = NBANKS * nt * (LTB // 128)
                    mb = msgp.tile([128, nch, F], BF16, tag="mb")
                    s0 = GBASE[g]
                    nidx = nt * LTB
                    for b in range(NBANKS):
                        c0 = (s0 + b * nidx) // 16
                        nc.gpsimd.dma_gather(
                            mb[:, b * (nidx // 128):(b + 1) * (nidx // 128), :],
                            bank_ap(b),
                            idx_t[:, c0:c0 + nidx // 16],
                            nidx, nidx, F, queue_num=qcnt[0] % 2)
                        qcnt[0] += 1
                    S_t = spool.tile([128, nch, 128], BF16, tag="S")
                    nc.vector.tensor_tensor(
                        S_t[:],
                        dloc_t[:, s0 // 128:s0 // 128 + nch]
                            .unsqueeze(-1).broadcast_to([128, nch, 128]),
                        iota_t[:].unsqueeze(1).broadcast_to([128, nch, 128]),
                        mybir.AluOpType.is_equal)
                    for ti in range(nt):
                        t = t0 + ti
                        gps = ppa.tile([128, F], F32, tag="gps")
                        for b in range(NBANKS):
                            for k in range(LTB // 128):
                                ch = b * (nt * LTB // 128) + ti * (LTB // 128) + k
                                nc.tensor.matmul(
                                    gps[:], S_t[:, ch, :], mb[:, ch, :],
                                    start=(b == 0 and k == 0),
                                    stop=(b == NBANKS - 1 and k == LTB // 128 - 1))
                        t1 = ep.tile([128, F], BF16, tag="t1")
                        nc.vector.tensor_copy(t1[:], gps[:])
                        nc.vector.tensor_tensor(t1[:], t1[:], self_t[:, t, :],
                                                mybir.AluOpType.add)
                        g2 = ep.tile([128, F], BF16, tag="g2")
                        nc.vector.tensor_tensor(
                            g2[:], t1[:],
                            dis_t[:, t:t + 1].broadcast_to([128, F]),
                            mybir.AluOpType.mult)
                        out_cb(t, g2)

            # ----- layer 1: table q~, epilogue -> h~ shard + allgather
            def l1_out(t, g2):
                t3 = ep.tile([128, F], BF16, tag="t3")
                nc.vector.tensor_tensor(t3[:], g2[:], b1_t[:],
                                        mybir.AluOpType.add)
                hr = ep.tile([128, F], BF16, tag="hr")
                nc.scalar.activation(hr[:], t3[:],
                                     mybir.ActivationFunctionType.Relu,
                                     bias=zb[:])
                nc.vector.tensor_tensor(
                    hself[:, t, :], hr[:],
                    dis_t[:, t:t + 1].broadcast_to([128, F]),
                    mybir.AluOpType.mult)
                nc.sync.dma_start(h_shard[t * 128:(t + 1) * 128, :],
                                  hself[:, t, :])

            agg_layer(lambda b: q_full[b * BANK:(b + 1) * BANK, :],
                      qself, l1_out)

            nc.gpsimd.collective_compute(
                "AllGather", mybir.AluOpType.bypass, replica_groups=groups,
                ins=[h_shard[:].opt()], outs=[h_full[:].opt()])

            # ----- layer 2: table h~, epilogue -> transpose, @W2, +b2
            def l2_out(t, g2):
                tps = ppt.tile([128, 128], BF16, tag="tps")
                nc.tensor.transpose(tps[:], g2[:], ident[:])
                g2T = ep.tile([128, 128], BF16, tag="g2T")
                nc.vector.tensor_copy(g2T[:], tps[:])
                ops = ppt.tile([128, 128], F32, tag="ops")
                nc.tensor.matmul(ops[:], w2_t[:], g2T[:], start=True, stop=True)
                ot = ep.tile([128, 128], F32, tag="ot")
                nc.vector.tensor_tensor(ot[:], ops[:],
                                        b2_t[:].broadcast_to([128, 128]),
                                        mybir.AluOpType.add)
                nc.sync.dma_start(outT[:, t * 128:(t + 1) * 128], ot[:])

            agg_layer(lambda b: h_full[b * BANK:(b + 1) * BANK, :],
                      hself, l2_out)

    nc.compile()
    return nc


class _SpmdRunner:
    """Persistent PJRT runner mirroring bass2jax.run_bass_via_pjrt's
    multi-core path, built once so repeated kernel() calls reuse the
    compiled executable."""

    def __init__(self, nc, n_cores):
        import jax
        from jax.sharding import Mesh, PartitionSpec
        from jax.experimental.shard_map import shard_map
        from concourse import bass2jax, mybir

        bass2jax.install_neuronx_cc_hook()
        self.jax = jax
        self.n_cores = n_cores
        partition_name = (nc.partition_id_tensor.name
                          if nc.partition_id_tensor else None)
        in_names, out_names, out_avals, zero_outs = [], [], [], []
        for alloc in nc.m.functions[0].allocations:
            if not isinstance(alloc, mybir.MemoryLocationSet):
                continue
            if not alloc.memorylocations:
                continue
            name = alloc.memorylocations[0].name
            if alloc.kind == "ExternalInput":
                if name != partition_name:
                    in_names.append(name)
            elif alloc.kind == "ExternalOutput":
                out_names.append(name)
                shape = tuple(alloc.tensor_shape)
                dtype = mybir.dt.np(alloc.dtype)
                out_avals.append(jax.core.ShapedArray(shape, dtype))
                zero_outs.append(np.zeros(shape, dtype))
        self.in_names, self.out_names = in_names, out_names
        self.out_avals, self.zero_outs = out_avals, zero_outs
        n_params = len(in_names)
        n_outs = len(out_avals)
        all_in = list(in_names) + list(out_names)
        if partition_name is not None:
            all_in.append(partition_name)
        donate = tuple(range(n_params, n_params + n_outs))

        def _body(*args):
            operands = list(args)
            if partition_name is not None:
                operands.append(bass2jax.partition_id_tensor())
            outs = bass2jax._bass_exec_p.bind(
                *operands,
                out_avals=tuple(out_avals),
                in_names=tuple(all_in),
                out_names=tuple(out_names),
                lowering_input_output_aliases=(),
                sim_require_finite=True,
                sim_require_nnan=True,
                nc=nc,
            )
            return tuple(outs)

        devices = jax.devices()[:n_cores]
        mesh = Mesh(np.asarray(devices), ("core",))
        in_specs = (PartitionSpec("core"),) * (n_params + n_outs)
        out_specs = (PartitionSpec("core"),) * len(out_names)
        self.fn = jax.jit(
            shard_map(_body, mesh=mesh, in_specs=in_specs,
                      out_specs=out_specs, check_rep=False),
            donate_argnums=donate, keep_unused=True)

    def run(self, in_maps):
        concat = [np.concatenate(
            [np.asarray(in_maps[c][name]) for c in range(self.n_cores)],
            axis=0) for name in self.in_names]
        zeros = [np.zeros((self.n_cores * z.shape[0], *z.shape[1:]), z.dtype)
                 for z in self.zero_outs]
        out = self.fn(*concat, *zeros)
        self.jax.block_until_ready(out)
        return [{name: np.asarray(out[i]).reshape(
                    self.n_cores, *self.out_avals[i].shape)[c]
                 for i, name in enumerate(self.out_names)}
                for c in range(self.n_cores)]


def _get_runner():
    if "runner" not in _CACHE:
        nc = _build_device()
        _CACHE["nc"] = nc
        _CACHE["runner"] = _SpmdRunner(nc, NCORES)
    return _CACHE["runner"]


# ---------------------------------------------------------------------------
# host-side index preparation
# ---------------------------------------------------------------------------

def _prep(edge_index):
    """Returns (perm, pos, dis, per-core metadata arrays).

    perm[i] = original node id at permuted position i; pos = inverse.
    The permutation keeps every node inside its original quarter
    (= gather bank), so bank assignments are permutation-independent.
    Within each quarter, nodes are dealt snake-wise by in-degree across
    the quarter's 196 tiles to balance (tile, src-bank) group loads.
    """
    row = np.asarray(edge_index[0], dtype=np.int64)
    col = np.asarray(edge_index[1], dtype=np.int64)

    deg = np.bincount(col, minlength=NP).astype(np.float32) + 1.0
    dis = 1.0 / np.sqrt(deg)

    src_bank = col // BANK  # fixed: sources keep their quarter

    # in-degree per node (for balancing)
    indeg = np.bincount(row, minlength=NP)

    perm = np.empty(NP, dtype=np.int64)
    TPQ = 2 * T  # tiles per quarter (2 cores per bank-quarter)
    for q in range(NBANKS):
        nodes = np.arange(q * BANK, (q + 1) * BANK)
        order = np.argsort(-indeg[nodes], kind="stable")
        nodes = nodes[order]
        # snake deal across TPQ tiles
        k = np.arange(BANK)
        rnd, pos_in_rnd = k // TPQ, k % TPQ
        tile_of = np.where(rnd % 2 == 0, pos_in_rnd, TPQ - 1 - pos_in_rnd)
        slot_of = rnd
        dest_pos = q * BANK + tile_of * 128 + slot_of
        perm[dest_pos] = nodes
    pos = np.empty(NP, dtype=np.int64)
    pos[perm] = np.arange(NP)

    prow = pos[row]           # permuted dest positions
    pcol = pos[col]           # permuted src positions
    core = prow // SHARD
    tilec = (prow % SHARD) // 128

    # group key (core, tile, bank) and slot packing
    gkey = (core * T + tilec) * NBANKS + src_bank
    loads = np.bincount(gkey, minlength=NCORES * T * NBANKS)
    mx = loads.max()
    if mx > LTB:
        raise RuntimeError(f"gather budget overflow: max group load {mx} > {LTB}")

    order = np.argsort(gkey, kind="stable")
    e_sorted = order
    gk_sorted = gkey[order]
    # rank within group
    starts = np.zeros(NCORES * T * NBANKS + 1, dtype=np.int64)
    np.cumsum(loads, out=starts[1:])
    rank = np.arange(E) - starts[gk_sorted]

    # slot id within the core: layout [group g][bank b][tile-in-group][LTB]
    tile_of = gk_sorted // NBANKS % T
    t_in_g = tile_of % G
    g_of = tile_of // G
    b_of = gk_sorted % NBANKS
    nt_of = np.minimum(G, T - g_of * G)
    gbase = np.asarray(GBASE[:-1], dtype=np.int64)
    slot = gbase[g_of] + b_of * (nt_of * LTB) + t_in_g * LTB + rank
    core_of = gk_sorted // (T * NBANKS)

    idx_arr = np.zeros((NCORES, 16, SLOTS // 16), dtype=np.int16)
    dl_arr = np.full((NCORES, 128, SLOTS // 128), 255.0, dtype=BF)

    idx_val = (pcol[e_sorted] - src_bank[e_sorted] * BANK).astype(np.int16)
    dl_val = (prow[e_sorted] % 128).astype(np.float32)
    idx_arr[core_of, slot % 16, slot // 16] = idx_val
    dl_arr[core_of, slot % 128, slot // 128] = dl_val.astype(BF)

    dis_perm = dis[perm]      # dis by permuted position
    return perm, pos, dis, dis_perm, idx_arr, dl_arr


def kernel(x, W1, b1, W2, b2, edge_index):
    x = np.asarray(x, dtype=np.float32)
    W1 = np.asarray(W1, dtype=np.float32)
    b1 = np.asarray(b1, dtype=np.float32)
    W2 = np.asarray(W2, dtype=np.float32)
    b2 = np.asarray(b2, dtype=np.float32)
    edge_index = np.asarray(edge_index)

    perm, pos, dis, dis_perm, idx_arr, dl_arr = _prep(edge_index)

    xp = np.zeros((NP, IN), dtype=np.float32)
    xp[:N] = x
    xp = xp[perm]             # permuted node order

    in_maps = []
    for c in range(NCORES):
        sl = slice(c * SHARD, (c + 1) * SHARD)
        in_maps.append({
            "xT": np.ascontiguousarray(xp[sl].T).astype(BF),
            "W1": W1.astype(BF),
            "W2": W2.astype(BF),
            "b1b": np.tile(b1[None, :], (128, 1)).astype(BF),
            "b2c": b2[:, None].astype(np.float32),
            "disc": np.ascontiguousarray(
                dis_perm[sl].reshape(T, 128).T).astype(np.float32),
            "idx": idx_arr[c],
            "dloc": dl_arr[c],
        })

    r = _get_runner()
    res = r.run(in_maps)

    full = np.concatenate([res[c]["outT"] for c in range(NCORES)], axis=1)
    return np.ascontiguousarray(full.T[pos[:N]]).astype(np.float32)
